# revision 9
# baseline (speedup 1.0000x reference)
"""Trainium2 Bass kernel for nn_DependencyEncoder3 (tree-LSTM dependency encoder).

Model: perfect 8-ary tree, 6 levels, level-order indexing, D=512, P=8 dep types.
  leaves: z = final cell state of a 1-step LSTM over [x]  (c = sig(i)*tanh(g))
  level l: per node, children codes are dep-transformed (z_c -> W_dep[p] z_c +
  b_dep[p]) and fed as a 9-step sequence [zc_0..zc_7, x_own] through the LSTM;
  z = final c.  Output: root z, [1, 512].

Sharding: core c owns the entire subtree of level-1 node c.  Levels 5..1 are
fully core-local; the only communication is a 16 KB AllGather of the eight
level-1 codes before the (replicated) root computation.

Per-core compute layout: activations are feature-major ([128 part, 4 chunks,
cols]); all matmuls keep weights stationary (lhsT) and stream node columns.
Per-dep weight selection is realized by sorting children by dep type: host
passes gather indices; GPSIMD indirect_copy does the SBUF column gathers
(children -> dep-sorted groups for the dep matmul; sorted positions -> step
order for the recurrence).  Matmuls run in bf16 (fp32 PSUM accumulate); cell
state c stays fp32.
"""

import sys

for _p in ("/opt/trn_rl_repo", "/root/.axon_site/_ro/trn_rl_repo"):
    if _p not in sys.path:
        sys.path.append(_p)

import numpy as np
import ml_dtypes
from contextlib import ExitStack

import concourse.bass as bass
import concourse.tile as tile
from concourse import bacc, mybir
from concourse.bass_utils import run_bass_kernel_spmd

F32 = mybir.dt.float32
BF16 = mybir.dt.bfloat16
U16 = mybir.dt.uint16
AFT = mybir.ActivationFunctionType
BF = ml_dtypes.bfloat16

NCORES = 8
D = 512
KC = 4            # feature chunks of 128
G = 2048          # gate width
P = 8             # dep types
K = 8             # children per node
OFF = [0, 1, 9, 73, 585, 4681, 37449]
LEAF = 4096       # leaves per core

# (name, C=children per core, m=nodes per core, xoff into xown cols, is_root)
LEVEL_SHAPES = [
    ("L4", 4096, 512, 0, False),
    ("L3", 512, 64, 512, False),
    ("L2", 64, 8, 576, False),
    ("L1", 8, 1, 584, False),
    ("RT", 8, 1, 585, True),
]
XOWN_COLS = 586


def _ceil16(x):
    return -(-x // 16) * 16


def _ceil32(x):
    # indirect_copy idx slices must be 4B-aligned: widths in 32-index units
    return -(-x // 32) * 32


def _wrap_idx(vals, ncols):
    """Wrap indices into the GPSIMD per-16-partition layout, replicated to 128
    partitions: idx[p, s] = vals[s*16 + (p % 16)]."""
    arr = np.zeros((16, ncols), dtype=np.uint16)
    v = np.asarray(vals, dtype=np.uint16)
    n = len(v)
    pad = np.zeros(ncols * 16, dtype=np.uint16)
    pad[:n] = v
    arr[:, :] = pad.reshape(ncols, 16).T
    return np.tile(arr, (8, 1))


def _broadcast_cols(ap, w):
    """Append a stride-0 dim of size w to a 2D sbuf AP slice [128, k]."""
    return bass.AP(tensor=ap.tensor, offset=ap.offset, ap=list(ap.ap) + [[0, w]])


def _chunks(total, step):
    out = []
    o = 0
    while o < total:
        out.append((o, min(step, total - o)))
        o += step
    return out


def build_program(caps):
    """Build the uniform SPMD program.  caps: per-level dep-group capacity."""
    nc = bacc.Bacc("TRN2", debug=False, num_devices=NCORES)

    xleaf_p = nc.declare_dram_parameter("xleaf", [D, LEAF], BF16, isOutput=False)
    xown_p = nc.declare_dram_parameter("xown", [D, XOWN_COLS], BF16, isOutput=False)
    wih_p = nc.declare_dram_parameter("wih_t", [D, G], BF16, isOutput=False)
    whh_p = nc.declare_dram_parameter("whh_t", [D, G], BF16, isOutput=False)
    wdep_p = nc.declare_dram_parameter("wdep_t", [P, D, D], BF16, isOutput=False)
    bsum_p = nc.declare_dram_parameter("bsum_t", [128, 16], F32, isOutput=False)
    bdep_p = nc.declare_dram_parameter("bdep_t", [128, 32], F32, isOutput=False)

    giw = [P * caps[i] // 16 for i in range(len(LEVEL_SHAPES))]
    siw = [K * _ceil32(m) // 16 for (_, _, m, _, _) in LEVEL_SHAPES]
    idxg_p = nc.declare_dram_parameter("idxg", [128, sum(giw)], U16, isOutput=False)
    idxs_p = nc.declare_dram_parameter("idxs", [128, sum(siw)], U16, isOutput=False)
    out_p = nc.declare_dram_parameter("out", [1, D], F32, isOutput=True)

    with ExitStack() as ctx:
        tc = ctx.enter_context(tile.TileContext(nc))
        wpool = ctx.enter_context(tc.tile_pool(name="w", bufs=1))
        zpool = ctx.enter_context(tc.tile_pool(name="z", bufs=1))
        gpool = ctx.enter_context(tc.tile_pool(name="g", bufs=2))
        npool = ctx.enter_context(tc.tile_pool(name="nl", bufs=4))
        dram = ctx.enter_context(tc.tile_pool(name="dram", bufs=1, space="DRAM"))

        # ---- persistent weights / indices ----
        wih = wpool.tile([128, KC, G], BF16)
        nc.sync.dma_start(out=wih, in_=wih_p.ap().rearrange("(c p) g -> p c g", p=128))
        whh = wpool.tile([128, KC, G], BF16)
        nc.sync.dma_start(out=whh, in_=whh_p.ap().rearrange("(c p) g -> p c g", p=128))
        xown = wpool.tile([128, KC, XOWN_COLS], BF16)
        nc.sync.dma_start(
            out=xown, in_=xown_p.ap().rearrange("(c p) n -> p c n", p=128)
        )
        bsum = wpool.tile([128, 16], F32)
        nc.sync.dma_start(out=bsum, in_=bsum_p.ap())
        bdep = wpool.tile([128, 32], F32)
        nc.sync.dma_start(out=bdep, in_=bdep_p.ap())
        idxg = wpool.tile([128, sum(giw)], U16)
        nc.sync.dma_start(out=idxg, in_=idxg_p.ap())
        idxs = wpool.tile([128, sum(siw)], U16)
        nc.sync.dma_start(out=idxs, in_=idxs_p.ap())

        # ---- leaf stage: z5 = sigmoid(i) * tanh(g), gates from x @ W_ih.T ----
        pz5_stack = ExitStack()
        pz5 = pz5_stack.enter_context(tc.tile_pool(name="pz5", bufs=1))
        z5 = pz5.tile([128, KC, LEAF], BF16, tag="z5")
        with tc.tile_pool(name="psl", bufs=4, space="PSUM") as psl:
            for nb in range(LEAF // 512):
                sl = slice(nb * 512, (nb + 1) * 512)
                xt = gpool.tile([128, KC, 512], BF16, tag="zct")
                nc.sync.dma_start(
                    out=xt,
                    in_=xleaf_p.ap().rearrange("(c p) n -> p c n", p=128)[:, :, sl],
                )
                for r in range(KC):
                    pi = psl.tile([128, 512], F32, tag="pleaf")
                    for kb in range(KC):
                        nc.tensor.matmul(
                            pi, wih[:, kb, r * 128:(r + 1) * 128], xt[:, kb, :],
                            start=(kb == 0), stop=(kb == KC - 1),
                        )
                    pg = psl.tile([128, 512], F32, tag="pleaf")
                    mmg = 8 + r
                    for kb in range(KC):
                        nc.tensor.matmul(
                            pg, wih[:, kb, mmg * 128:(mmg + 1) * 128], xt[:, kb, :],
                            start=(kb == 0), stop=(kb == KC - 1),
                        )
                    si = npool.tile([128, 512], F32, tag="lnl")
                    nc.scalar.activation(
                        out=si, in_=pi, func=AFT.Sigmoid, bias=bsum[:, r:r + 1]
                    )
                    tg = npool.tile([128, 512], F32, tag="lnl")
                    nc.scalar.activation(
                        out=tg, in_=pg, func=AFT.Tanh, bias=bsum[:, mmg:mmg + 1]
                    )
                    nc.vector.tensor_mul(z5[:, r, sl], si, tg)

        # ---- level sweep ----
        h = zpool.tile([128, KC, 512], BF16, tag="h", name="h")
        c = zpool.tile([128, KC, 512], F32, tag="c", name="c")
        goff = 0
        soff = 0
        zin = z5
        c_final = None
        for li, (lname, C, m, xoff, is_root) in enumerate(LEVEL_SHAPES):
            cap = caps[li]
            Cs = P * cap
            mw = _ceil32(m)

            if is_root:
                # AllGather the eight level-1 codes; every core computes the
                # true root (identical inputs everywhere).
                cc_in = dram.tile([1, D], F32)
                nc.gpsimd.dma_start(
                    out=cc_in.rearrange("a (c p) -> p c a", p=128),
                    in_=zin[:, :, 0:1],
                )
                cc_out = dram.tile([NCORES, D], F32)
                nc.gpsimd.collective_compute(
                    "AllGather",
                    mybir.AluOpType.bypass,
                    replica_groups=[list(range(NCORES))],
                    ins=[cc_in.opt()],
                    outs=[cc_out.opt()],
                )
                zrt = zpool.tile([128, KC, K], BF16, tag="zrt")
                for r in range(KC):
                    nc.gpsimd.dma_start(
                        out=zrt[:, r, :],
                        in_=cc_out.rearrange("n (c p) -> p c n", p=128)[:, r, :],
                    )
                zin = zrt

            # --- dep transform: zcs[:, :, p*cap + j] = Wdep[p] @ z_sorted + bdep[p]
            lvl_stack = ExitStack()
            pgin = (None if m >= 256 else
                    lvl_stack.enter_context(
                        tc.tile_pool(name=f"pgin{li}", bufs=1)))
            zcs = zpool.tile([128, KC, max(Cs, 128)], BF16, tag="zcs")
            with tc.tile_pool(name=f"psd{li}", bufs=3, space="PSUM") as psd:
                for p in range(P):
                    wdp = gpool.tile([128, KC, D], BF16, tag="wdp")
                    nc.sync.dma_start(
                        out=wdp,
                        in_=wdep_p.ap().rearrange(
                            "d (c p) m -> d p c m", p=128)[p],
                    )
                    zs = gpool.tile([128, KC, max(caps)], BF16, tag="zs")
                    ig = idxg[:, goff + p * cap // 16: goff + (p + 1) * cap // 16]
                    for r in range(KC):
                        nc.gpsimd.indirect_copy(
                            out=zs[:, r, 0:cap], data=zin[:, r, 0:C], idxs=ig,
                            i_know_ap_gather_is_preferred=True,
                        )
                    for (n0, w) in _chunks(cap, 512):
                        for mmr in range(KC):
                            pd = psd.tile([128, 512], F32, tag="pd")
                            for kb in range(KC):
                                nc.tensor.matmul(
                                    pd[:, 0:w],
                                    wdp[:, kb, mmr * 128:(mmr + 1) * 128],
                                    zs[:, kb, n0:n0 + w],
                                    start=(kb == 0), stop=(kb == KC - 1),
                                )
                            nc.scalar.activation(
                                out=zcs[:, mmr, p * cap + n0: p * cap + n0 + w],
                                in_=pd[:, 0:w],
                                func=AFT.Identity,
                                bias=bdep[:, mmr * 8 + p: mmr * 8 + p + 1],
                            )

                big = m >= 256
                gin = None
                zclv = None
                if big:
                    pass
                elif True:
                    # pre-gather step-ordered children and batch the whole
                    # input projection (incl. biases) into Gin [128, 16, 9m]
                    sw = mw
                    zclv = pgin.tile([128, KC, 9 * sw], BF16,
                                     tag="zclv", name="zclv")
                    nc.vector.memset(zclv, 0.0)
                    for t in range(K):
                        isl = idxs[:, soff + t * mw // 16: soff + (t + 1) * mw // 16]
                        for r in range(KC):
                            nc.gpsimd.indirect_copy(
                                out=zclv[:, r, t * sw:t * sw + m],
                                data=zcs[:, r, :],
                                idxs=isl,
                                i_know_ap_gather_is_preferred=True,
                            )
                    nc.vector.tensor_copy(
                        out=zclv[:, :, 8 * sw:8 * sw + m],
                        in_=xown[:, :, xoff:xoff + m],
                    )
                    gin = pgin.tile([128, 16, 9 * sw], F32,
                                    tag="gin", name="gin")
                    for (n0, w) in _chunks(9 * sw, 512):
                        for mm in range(16):
                            pd = psd.tile([128, 512], F32, tag="pd")
                            for kb in range(KC):
                                nc.tensor.matmul(
                                    pd[:, 0:w],
                                    wih[:, kb, mm * 128:(mm + 1) * 128],
                                    zclv[:, kb, n0:n0 + w],
                                    start=(kb == 0), stop=(kb == KC - 1),
                                )
                            nc.scalar.activation(
                                out=gin[:, mm, n0:n0 + w], in_=pd[:, 0:w],
                                func=AFT.Identity, bias=bsum[:, mm:mm + 1],
                            )

            if li == 0:
                pz5_stack.close()

            # --- recurrence ---
            if is_root:
                zout = None
            else:
                ztag = "za" if li % 2 == 0 else "zb"
                zout = zpool.tile([128, KC, max(m, 16)], BF16, tag=ztag)

            if big:
                with tc.tile_pool(name=f"psg{li}", bufs=1, space="PSUM") as psg:
                    for t in range(K + 1):
                        if t < K:
                            zct = gpool.tile([128, KC, m], BF16, tag="zct")
                            isl = idxs[:, soff + t * mw // 16:
                                       soff + (t + 1) * mw // 16]
                            for r in range(KC):
                                nc.gpsimd.indirect_copy(
                                    out=zct[:, r, :], data=zcs[:, r, :], idxs=isl,
                                    i_know_ap_gather_is_preferred=True,
                                )
                            rhs_in = zct
                        else:
                            rhs_in = xown[:, :, xoff:xoff + m]
                        for (h0, w) in _chunks(m, 256):
                            pgA = psg.tile([128, 8, 256], F32, tag="pgA")
                            pgB = psg.tile([128, 8, 256], F32, tag="pgB")
                            for mm in range(16):
                                if t == 0 and 4 <= mm < 8:
                                    continue  # f unused at t=0
                                if t == K and mm >= 12:
                                    continue  # o unused at final step
                                tgt = (pgA[:, mm, 0:w] if mm < 8
                                       else pgB[:, mm - 8, 0:w])
                                for kb in range(KC):
                                    nc.tensor.matmul(
                                        tgt, wih[:, kb, mm * 128:(mm + 1) * 128],
                                        rhs_in[:, kb, h0:h0 + w],
                                        start=(kb == 0),
                                        stop=(t == 0 and kb == KC - 1),
                                    )
                                if t > 0:
                                    for kb in range(KC):
                                        nc.tensor.matmul(
                                            tgt, whh[:, kb, mm * 128:(mm + 1) * 128],
                                            h[:, kb, h0:h0 + w],
                                            start=False, stop=(kb == KC - 1),
                                        )
                            if t == 0:
                                nc.vector.tensor_add(
                                    pgA[:, 0:4, :], pgA[:, 0:4, :],
                                    _broadcast_cols(bsum[:, 0:4], 256),
                                )
                            else:
                                nc.vector.tensor_add(
                                    pgA, pgA, _broadcast_cols(bsum[:, 0:8], 256)
                                )
                            if t == K:
                                nc.vector.tensor_add(
                                    pgB[:, 0:4, :], pgB[:, 0:4, :],
                                    _broadcast_cols(bsum[:, 8:12], 256),
                                )
                            else:
                                nc.vector.tensor_add(
                                    pgB, pgB, _broadcast_cols(bsum[:, 8:16], 256)
                                )
                            si = npool.tile([128, KC, 256], F32, tag="nl")
                            nc.scalar.activation(
                                out=si, in_=pgA[:, 0:4, :], func=AFT.Sigmoid
                            )
                            if t > 0:
                                fs = npool.tile([128, KC, 256], F32, tag="nl")
                                nc.scalar.activation(
                                    out=fs, in_=pgA[:, 4:8, :], func=AFT.Sigmoid
                                )
                            if t < K:
                                os_ = npool.tile([128, KC, 256], F32, tag="nl")
                                nc.scalar.activation(
                                    out=os_, in_=pgB[:, 4:8, :], func=AFT.Sigmoid
                                )
                            gt = npool.tile([128, KC, 256], F32, tag="nl")
                            nc.scalar.activation(
                                out=gt, in_=pgB[:, 0:4, :], func=AFT.Tanh
                            )
                            csl = c[:, :, h0:h0 + w]
                            if t == 0:
                                nc.vector.tensor_mul(csl, si, gt)
                            else:
                                nc.vector.tensor_mul(csl, fs, csl)
                                tmp = npool.tile([128, KC, 256], F32, tag="nl")
                                nc.vector.tensor_mul(tmp, si, gt)
                                nc.vector.tensor_add(csl, csl, tmp)
                            if t < K:
                                tcv = npool.tile([128, KC, 256], F32, tag="nl")
                                nc.scalar.activation(out=tcv, in_=csl, func=AFT.Tanh)
                                nc.vector.tensor_mul(h[:, :, h0:h0 + w], os_, tcv)
                            else:
                                nc.vector.tensor_copy(
                                    out=zout[:, :, h0:h0 + w], in_=csl
                                )
            else:
                with tc.tile_pool(name=f"psg{li}", bufs=2, space="PSUM") as psg:
                    for t in range(K + 1):
                        if t > 0:
                            pg = psg.tile([128, 16, max(m, 16)], F32, tag="pgs")
                            for mm in range(16):
                                if t == K and mm >= 12:
                                    continue
                                for kb in range(KC):
                                    nc.tensor.matmul(
                                        pg[:, mm, 0:m],
                                        whh[:, kb, mm * 128:(mm + 1) * 128],
                                        h[:, kb, 0:m],
                                        start=(kb == 0), stop=(kb == KC - 1),
                                    )
                            gm = 12 if t == K else 16
                            nc.vector.tensor_add(
                                pg[:, 0:gm, 0:m], pg[:, 0:gm, 0:m],
                                gin[:, 0:gm, t * mw:t * mw + m],
                            )
                            gsrc = pg
                            gw = max(m, 16)
                        else:
                            gsrc = gin
                            gw = 9 * max(m, 16)

                        si = npool.tile([128, KC, max(m, 16)], F32, tag="snl")
                        nc.scalar.activation(
                            out=si[:, :, 0:m], in_=gsrc[:, 0:4, 0:m],
                            func=AFT.Sigmoid,
                        )
                        if t > 0:
                            fs = npool.tile([128, KC, max(m, 16)], F32, tag="snl")
                            nc.scalar.activation(
                                out=fs[:, :, 0:m], in_=gsrc[:, 4:8, 0:m],
                                func=AFT.Sigmoid,
                            )
                        if t < K:
                            os_ = npool.tile([128, KC, max(m, 16)], F32, tag="snl")
                            nc.scalar.activation(
                                out=os_[:, :, 0:m], in_=gsrc[:, 12:16, 0:m],
                                func=AFT.Sigmoid,
                            )
                        gt = npool.tile([128, KC, max(m, 16)], F32, tag="snl")
                        nc.scalar.activation(
                            out=gt[:, :, 0:m], in_=gsrc[:, 8:12, 0:m], func=AFT.Tanh
                        )
                        csl = c[:, :, 0:m]
                        if t == 0:
                            nc.vector.tensor_mul(csl, si[:, :, 0:m], gt[:, :, 0:m])
                        else:
                            nc.vector.tensor_mul(csl, fs[:, :, 0:m], csl)
                            tmp = npool.tile([128, KC, max(m, 16)], F32, tag="snl")
                            nc.vector.tensor_mul(
                                tmp[:, :, 0:m], si[:, :, 0:m], gt[:, :, 0:m]
                            )
                            nc.vector.tensor_add(csl, csl, tmp[:, :, 0:m])
                        if t < K:
                            tcv = npool.tile([128, KC, max(m, 16)], F32, tag="snl")
                            nc.scalar.activation(
                                out=tcv[:, :, 0:m], in_=csl, func=AFT.Tanh
                            )
                            nc.vector.tensor_mul(
                                h[:, :, 0:m], os_[:, :, 0:m], tcv[:, :, 0:m]
                            )
                        elif not is_root:
                            nc.vector.tensor_copy(out=zout[:, :, 0:m], in_=csl)

            lvl_stack.close()
            if is_root:
                c_final = c
            zin = zout
            goff += giw[li]
            soff += siw[li]

        nc.sync.dma_start(
            out=out_p.ap().rearrange("a (c p) -> p c a", p=128),
            in_=c_final[:, :, 0:1],
        )

    nc.finalize()
    return nc


def _prep_core_inputs(core, embeddings, dep_types, wih_t_bf, whh_t_bf, wdep_t_bf,
                      bsum_t, bdep_t, caps):
    emb = embeddings

    xleaf = np.ascontiguousarray(
        emb[OFF[5] + LEAF * core: OFF[5] + LEAF * (core + 1)].T
    ).astype(BF)

    xown_cols = []
    for (_, C, m, _, is_root) in LEVEL_SHAPES:
        if is_root:
            xown_cols.append(emb[0:1])
        else:
            lvl = {512: 4, 64: 3, 8: 2, 1: 1}[m]
            s = OFF[lvl] + m * core
            xown_cols.append(emb[s:s + m])
    xown = np.ascontiguousarray(np.concatenate(xown_cols, axis=0).T).astype(BF)

    giw_cols = []
    siw_cols = []
    for li, (_, C, m, _, is_root) in enumerate(LEVEL_SHAPES):
        cap = caps[li]
        mw = _ceil32(m)
        if is_root:
            deps = dep_types[1:9]
        else:
            lvl = {512: 4, 64: 3, 8: 2, 1: 1}[m]
            s = OFF[lvl + 1] + C * core
            deps = dep_types[s:s + C]
        pos_of_child = np.zeros(C, dtype=np.int64)
        for p in range(P):
            idx_p = np.where(deps == p)[0]
            pos_of_child[idx_p] = p * cap + np.arange(len(idx_p))
            giw_cols.append(_wrap_idx(idx_p, cap // 16))
        for t in range(K):
            child = np.arange(m) * K + t
            siw_cols.append(_wrap_idx(pos_of_child[child], mw // 16))
    idxg = np.concatenate(giw_cols, axis=1)
    idxs = np.concatenate(siw_cols, axis=1)

    return {
        "xleaf": xleaf,
        "xown": xown,
        "wih_t": wih_t_bf,
        "whh_t": whh_t_bf,
        "wdep_t": wdep_t_bf,
        "bsum_t": bsum_t,
        "bdep_t": bdep_t,
        "idxg": idxg,
        "idxs": idxs,
    }


_CACHED = {}


def kernel(embeddings, dep_types, W_dep, b_dep, W_ih, W_hh, b_ih, b_hh):
    embeddings = np.asarray(embeddings, dtype=np.float32)
    dep_types = np.asarray(dep_types)
    W_dep = np.asarray(W_dep, dtype=np.float32)
    b_dep = np.asarray(b_dep, dtype=np.float32)
    W_ih = np.asarray(W_ih, dtype=np.float32)
    W_hh = np.asarray(W_hh, dtype=np.float32)
    b_ih = np.asarray(b_ih, dtype=np.float32)
    b_hh = np.asarray(b_hh, dtype=np.float32)

    # per-level dep-group capacities (max group size over cores, ceil to 16)
    caps = []
    for (_, C, m, _, is_root) in LEVEL_SHAPES:
        if is_root:
            mx = int(np.bincount(dep_types[1:9], minlength=P).max())
        else:
            lvl = {512: 4, 64: 3, 8: 2, 1: 1}[m]
            mx = 0
            for c in range(NCORES):
                s = OFF[lvl + 1] + C * c
                mx = max(mx, int(np.bincount(dep_types[s:s + C],
                                             minlength=P).max()))
        caps.append(max(_ceil32(mx), 32))

    wih_t_bf = np.ascontiguousarray(W_ih.T).astype(BF)
    whh_t_bf = np.ascontiguousarray(W_hh.T).astype(BF)
    wdep_t_bf = np.ascontiguousarray(W_dep.transpose(0, 2, 1)).astype(BF)
    bsum = (b_ih + b_hh).astype(np.float32)
    bsum_t = np.ascontiguousarray(bsum.reshape(16, 128).T)
    # bdep[:, r*8 + p] = b_dep[p, r*128:(r+1)*128]
    bdep_t = np.ascontiguousarray(
        b_dep.T.reshape(KC, 128, P).transpose(1, 0, 2).reshape(128, KC * P)
    )

    key = tuple(caps)
    if key not in _CACHED:
        _CACHED[key] = build_program(caps)
    nc = _CACHED[key]

    in_maps = [
        _prep_core_inputs(c, embeddings, dep_types, wih_t_bf, whh_t_bf,
                          wdep_t_bf, bsum_t, bdep_t, caps)
        for c in range(NCORES)
    ]
    res = run_bass_kernel_spmd(nc, in_maps, list(range(NCORES)))
    out = np.asarray(res.results[0]["out"], dtype=np.float32).reshape(1, D)
    return out


# revision 10
# speedup vs baseline: 1.2535x; 1.2535x over previous
"""Trainium2 Bass kernel for nn_DependencyEncoder3 (tree-LSTM dependency encoder).

Model: perfect 8-ary tree, 6 levels, level-order indexing, D=512, P=8 dep types.
  leaves: z = final cell state of a 1-step LSTM over [x]  (c = sig(i)*tanh(g))
  level l: per node, children codes are dep-transformed (z_c -> W_dep[p] z_c +
  b_dep[p]) and fed as a 9-step sequence [zc_0..zc_7, x_own] through the LSTM;
  z = final c.  Output: root z, [1, 512].

Sharding: core c owns the entire subtree of level-1 node c.  Levels 5..1 are
fully core-local; the only communication is a 16 KB AllGather of the eight
level-1 codes before the (replicated) root computation.

Per-core compute layout: activations are feature-major ([128 part, 4 chunks,
cols]); all matmuls keep weights stationary (lhsT) and stream node columns.
Per-dep weight selection is realized by sorting children by dep type: host
passes gather indices; GPSIMD indirect_copy does the SBUF column gathers
(children -> dep-sorted groups for the dep matmul; sorted positions -> step
order for the recurrence).  Matmuls run in bf16 (fp32 PSUM accumulate); cell
state c stays fp32.
"""

import sys

for _p in ("/opt/trn_rl_repo", "/root/.axon_site/_ro/trn_rl_repo"):
    if _p not in sys.path:
        sys.path.append(_p)

import numpy as np
import ml_dtypes
from contextlib import ExitStack

import concourse.bass as bass
import concourse.tile as tile
from concourse import bacc, mybir
from concourse.bass_utils import run_bass_kernel_spmd

F32 = mybir.dt.float32
BF16 = mybir.dt.bfloat16
U16 = mybir.dt.uint16
AFT = mybir.ActivationFunctionType
BF = ml_dtypes.bfloat16

NCORES = 8
D = 512
KC = 4            # feature chunks of 128
G = 2048          # gate width
P = 8             # dep types
K = 8             # children per node
OFF = [0, 1, 9, 73, 585, 4681, 37449]
LEAF = 4096       # leaves per core

# (name, C=children per core, m=nodes per core, xoff into xown cols, is_root)
LEVEL_SHAPES = [
    ("L4", 4096, 512, 0, False),
    ("L3", 512, 64, 512, False),
    ("L2", 64, 8, 576, False),
    ("L1", 8, 1, 584, False),
    ("RT", 8, 1, 585, True),
]
XOWN_COLS = 586


def _ceil16(x):
    return -(-x // 16) * 16


def _ceil32(x):
    # indirect_copy idx slices must be 4B-aligned: widths in 32-index units
    return -(-x // 32) * 32


def _wrap_idx(vals, ncols):
    """Wrap indices into the GPSIMD per-16-partition layout, replicated to 128
    partitions: idx[p, s] = vals[s*16 + (p % 16)]."""
    arr = np.zeros((16, ncols), dtype=np.uint16)
    v = np.asarray(vals, dtype=np.uint16)
    n = len(v)
    pad = np.zeros(ncols * 16, dtype=np.uint16)
    pad[:n] = v
    arr[:, :] = pad.reshape(ncols, 16).T
    return np.tile(arr, (8, 1))


def _broadcast_cols(ap, w):
    """Append a stride-0 dim of size w to a 2D sbuf AP slice [128, k]."""
    return bass.AP(tensor=ap.tensor, offset=ap.offset, ap=list(ap.ap) + [[0, w]])


def _chunks(total, step):
    out = []
    o = 0
    while o < total:
        out.append((o, min(step, total - o)))
        o += step
    return out


def build_program(caps):
    """Build the uniform SPMD program.  caps: per-level dep-group capacity."""
    nc = bacc.Bacc("TRN2", debug=False, num_devices=NCORES)

    leaf_pad = P * caps[0]
    xleaf_p = nc.declare_dram_parameter("xleaf", [D, leaf_pad], BF16, isOutput=False)
    xown_p = nc.declare_dram_parameter("xown", [D, XOWN_COLS], BF16, isOutput=False)
    wih_p = nc.declare_dram_parameter("wih_t", [D, G], BF16, isOutput=False)
    whh_p = nc.declare_dram_parameter("whh_t", [D, G], BF16, isOutput=False)
    wdep_p = nc.declare_dram_parameter("wdep_t", [P, D, D], BF16, isOutput=False)
    bsum_p = nc.declare_dram_parameter("bsum_t", [128, 16], F32, isOutput=False)
    bdep_p = nc.declare_dram_parameter("bdep_t", [128, 32], F32, isOutput=False)

    giw = [0 if i == 0 else P * caps[i] // 16
           for i in range(len(LEVEL_SHAPES))]
    siw = [K * _ceil32(m) // 16 for (_, _, m, _, _) in LEVEL_SHAPES]
    idxg_p = nc.declare_dram_parameter("idxg", [128, sum(giw)], U16, isOutput=False)
    idxs_p = nc.declare_dram_parameter("idxs", [128, sum(siw)], U16, isOutput=False)
    out_p = nc.declare_dram_parameter("out", [1, D], F32, isOutput=True)

    with ExitStack() as ctx:
        tc = ctx.enter_context(tile.TileContext(nc))
        wpool = ctx.enter_context(tc.tile_pool(name="w", bufs=1))
        zpool = ctx.enter_context(tc.tile_pool(name="z", bufs=1))
        gpool = ctx.enter_context(tc.tile_pool(name="g", bufs=2))
        npool = ctx.enter_context(tc.tile_pool(name="nl", bufs=4))
        dram = ctx.enter_context(tc.tile_pool(name="dram", bufs=1, space="DRAM"))

        # ---- persistent weights / indices ----
        wih = wpool.tile([128, KC, G], BF16)
        nc.sync.dma_start(out=wih, in_=wih_p.ap().rearrange("(c p) g -> p c g", p=128))
        whh = wpool.tile([128, KC, G], BF16)
        nc.sync.dma_start(out=whh, in_=whh_p.ap().rearrange("(c p) g -> p c g", p=128))
        xown = wpool.tile([128, KC, XOWN_COLS], BF16)
        nc.sync.dma_start(
            out=xown, in_=xown_p.ap().rearrange("(c p) n -> p c n", p=128)
        )
        bsum = wpool.tile([128, 16], F32)
        nc.sync.dma_start(out=bsum, in_=bsum_p.ap())
        bdep = wpool.tile([128, 32], F32)
        nc.sync.dma_start(out=bdep, in_=bdep_p.ap())
        idxg = wpool.tile([128, sum(giw)], U16)
        nc.sync.dma_start(out=idxg, in_=idxg_p.ap())
        idxs = wpool.tile([128, sum(siw)], U16)
        nc.sync.dma_start(out=idxs, in_=idxs_p.ap())

        # ---- leaf stage: z5 = sigmoid(i) * tanh(g), gates from x @ W_ih.T ----
        pz5_stack = ExitStack()
        pz5 = pz5_stack.enter_context(tc.tile_pool(name="pz5", bufs=1))
        z5 = pz5.tile([128, KC, leaf_pad], BF16, tag="z5")
        with tc.tile_pool(name="psl", bufs=4, space="PSUM") as psl:
            for nb in range(leaf_pad // 512):
                sl = slice(nb * 512, (nb + 1) * 512)
                xt = gpool.tile([128, KC, 512], BF16, tag="zct")
                nc.sync.dma_start(
                    out=xt,
                    in_=xleaf_p.ap().rearrange("(c p) n -> p c n", p=128)[:, :, sl],
                )
                for r in range(KC):
                    pi = psl.tile([128, 512], F32, tag="pleaf")
                    for kb in range(KC):
                        nc.tensor.matmul(
                            pi, wih[:, kb, r * 128:(r + 1) * 128], xt[:, kb, :],
                            start=(kb == 0), stop=(kb == KC - 1),
                        )
                    pg = psl.tile([128, 512], F32, tag="pleaf")
                    mmg = 8 + r
                    for kb in range(KC):
                        nc.tensor.matmul(
                            pg, wih[:, kb, mmg * 128:(mmg + 1) * 128], xt[:, kb, :],
                            start=(kb == 0), stop=(kb == KC - 1),
                        )
                    si = npool.tile([128, 512], F32, tag="lnl")
                    nc.scalar.activation(
                        out=si, in_=pi, func=AFT.Sigmoid, bias=bsum[:, r:r + 1]
                    )
                    tg = npool.tile([128, 512], F32, tag="lnl")
                    nc.scalar.activation(
                        out=tg, in_=pg, func=AFT.Tanh, bias=bsum[:, mmg:mmg + 1]
                    )
                    nc.vector.tensor_mul(z5[:, r, sl], si, tg)

        # ---- level sweep ----
        h = zpool.tile([128, KC, 512], BF16, tag="h", name="h")
        c = zpool.tile([128, KC, 512], F32, tag="c", name="c")
        goff = 0
        soff = 0
        zin = z5
        c_final = None
        for li, (lname, C, m, xoff, is_root) in enumerate(LEVEL_SHAPES):
            cap = caps[li]
            Cs = P * cap
            mw = _ceil32(m)

            if is_root:
                # AllGather the eight level-1 codes; every core computes the
                # true root (identical inputs everywhere).
                cc_in = dram.tile([1, D], F32)
                nc.gpsimd.dma_start(
                    out=cc_in.rearrange("a (c p) -> p c a", p=128),
                    in_=zin[:, :, 0:1],
                )
                cc_out = dram.tile([NCORES, D], F32)
                nc.gpsimd.collective_compute(
                    "AllGather",
                    mybir.AluOpType.bypass,
                    replica_groups=[list(range(NCORES))],
                    ins=[cc_in.opt()],
                    outs=[cc_out.opt()],
                )
                zrt = zpool.tile([128, KC, K], BF16, tag="zrt")
                for r in range(KC):
                    nc.gpsimd.dma_start(
                        out=zrt[:, r, :],
                        in_=cc_out.rearrange("n (c p) -> p c n", p=128)[:, r, :],
                    )
                zin = zrt

            # --- dep transform: zcs[:, :, p*cap + j] = Wdep[p] @ z_sorted + bdep[p]
            lvl_stack = ExitStack()
            pgin = (None if m >= 256 else
                    lvl_stack.enter_context(
                        tc.tile_pool(name=f"pgin{li}", bufs=1)))
            zcs = zpool.tile([128, KC, max(Cs, 128)], BF16, tag="zcs")
            with tc.tile_pool(name=f"psd{li}", bufs=3, space="PSUM") as psd:
                for p in range(P):
                    wdp = gpool.tile([128, KC, D], BF16, tag="wdp")
                    nc.sync.dma_start(
                        out=wdp,
                        in_=wdep_p.ap().rearrange(
                            "d (c p) m -> d p c m", p=128)[p],
                    )
                    if li == 0:
                        # leaves arrive dep-sorted from the host: groups are
                        # plain column ranges of z5, no gather needed.
                        zs = zin[:, :, p * cap:(p + 1) * cap]
                    else:
                        zs = gpool.tile([128, KC, max(caps)], BF16, tag="zs")
                        ig = idxg[:, goff + p * cap // 16:
                                  goff + (p + 1) * cap // 16]
                        for r in range(KC):
                            nc.gpsimd.indirect_copy(
                                out=zs[:, r, 0:cap], data=zin[:, r, 0:C],
                                idxs=ig,
                                i_know_ap_gather_is_preferred=True,
                            )
                    for (n0, w) in _chunks(cap, 512):
                        for mmr in range(KC):
                            pd = psd.tile([128, 512], F32, tag="pd")
                            for kb in range(KC):
                                nc.tensor.matmul(
                                    pd[:, 0:w],
                                    wdp[:, kb, mmr * 128:(mmr + 1) * 128],
                                    zs[:, kb, n0:n0 + w],
                                    start=(kb == 0), stop=(kb == KC - 1),
                                )
                            nc.scalar.activation(
                                out=zcs[:, mmr, p * cap + n0: p * cap + n0 + w],
                                in_=pd[:, 0:w],
                                func=AFT.Identity,
                                bias=bdep[:, mmr * 8 + p: mmr * 8 + p + 1],
                            )

                big = m >= 256
                gin = None
                zclv = None
                if big:
                    pass
                elif True:
                    # pre-gather step-ordered children and batch the whole
                    # input projection (incl. biases) into Gin [128, 16, 9m]
                    sw = mw
                    zclv = pgin.tile([128, KC, 9 * sw], BF16,
                                     tag="zclv", name="zclv")
                    nc.vector.memset(zclv, 0.0)
                    for t in range(K):
                        isl = idxs[:, soff + t * mw // 16: soff + (t + 1) * mw // 16]
                        for r in range(KC):
                            nc.gpsimd.indirect_copy(
                                out=zclv[:, r, t * sw:t * sw + m],
                                data=zcs[:, r, :],
                                idxs=isl,
                                i_know_ap_gather_is_preferred=True,
                            )
                    nc.vector.tensor_copy(
                        out=zclv[:, :, 8 * sw:8 * sw + m],
                        in_=xown[:, :, xoff:xoff + m],
                    )
                    gin = pgin.tile([128, 16, 9 * sw], F32,
                                    tag="gin", name="gin")
                    for (n0, w) in _chunks(9 * sw, 512):
                        for mm in range(16):
                            pd = psd.tile([128, 512], F32, tag="pd")
                            for kb in range(KC):
                                nc.tensor.matmul(
                                    pd[:, 0:w],
                                    wih[:, kb, mm * 128:(mm + 1) * 128],
                                    zclv[:, kb, n0:n0 + w],
                                    start=(kb == 0), stop=(kb == KC - 1),
                                )
                            nc.scalar.activation(
                                out=gin[:, mm, n0:n0 + w], in_=pd[:, 0:w],
                                func=AFT.Identity, bias=bsum[:, mm:mm + 1],
                            )

            if li == 0:
                pz5_stack.close()

            # --- recurrence ---
            if is_root:
                zout = None
            else:
                ztag = "za" if li % 2 == 0 else "zb"
                zout = zpool.tile([128, KC, max(m, 16)], BF16, tag=ztag)

            if big:
                with tc.tile_pool(name=f"psg{li}", bufs=1, space="PSUM") as psg:
                    for t in range(K + 1):
                        if t < K:
                            zct = gpool.tile([128, KC, m], BF16, tag="zct")
                            isl = idxs[:, soff + t * mw // 16:
                                       soff + (t + 1) * mw // 16]
                            for r in range(KC):
                                nc.gpsimd.indirect_copy(
                                    out=zct[:, r, :], data=zcs[:, r, :], idxs=isl,
                                    i_know_ap_gather_is_preferred=True,
                                )
                            rhs_in = zct
                        else:
                            rhs_in = xown[:, :, xoff:xoff + m]
                        for (h0, w) in _chunks(m, 256):
                            pgA = psg.tile([128, 8, 256], F32, tag="pgA")
                            pgB = psg.tile([128, 8, 256], F32, tag="pgB")
                            for mm in range(16):
                                if t == 0 and 4 <= mm < 8:
                                    continue  # f unused at t=0
                                if t == K and mm >= 12:
                                    continue  # o unused at final step
                                tgt = (pgA[:, mm, 0:w] if mm < 8
                                       else pgB[:, mm - 8, 0:w])
                                for kb in range(KC):
                                    nc.tensor.matmul(
                                        tgt, wih[:, kb, mm * 128:(mm + 1) * 128],
                                        rhs_in[:, kb, h0:h0 + w],
                                        start=(kb == 0),
                                        stop=(t == 0 and kb == KC - 1),
                                    )
                                if t > 0:
                                    for kb in range(KC):
                                        nc.tensor.matmul(
                                            tgt, whh[:, kb, mm * 128:(mm + 1) * 128],
                                            h[:, kb, h0:h0 + w],
                                            start=False, stop=(kb == KC - 1),
                                        )
                            if t == 0:
                                nc.vector.tensor_add(
                                    pgA[:, 0:4, :], pgA[:, 0:4, :],
                                    _broadcast_cols(bsum[:, 0:4], 256),
                                )
                            else:
                                nc.vector.tensor_add(
                                    pgA, pgA, _broadcast_cols(bsum[:, 0:8], 256)
                                )
                            if t == K:
                                nc.vector.tensor_add(
                                    pgB[:, 0:4, :], pgB[:, 0:4, :],
                                    _broadcast_cols(bsum[:, 8:12], 256),
                                )
                            else:
                                nc.vector.tensor_add(
                                    pgB, pgB, _broadcast_cols(bsum[:, 8:16], 256)
                                )
                            si = npool.tile([128, KC, 256], F32, tag="nl")
                            nc.scalar.activation(
                                out=si, in_=pgA[:, 0:4, :], func=AFT.Sigmoid
                            )
                            if t > 0:
                                fs = npool.tile([128, KC, 256], F32, tag="nl")
                                nc.scalar.activation(
                                    out=fs, in_=pgA[:, 4:8, :], func=AFT.Sigmoid
                                )
                            if t < K:
                                os_ = npool.tile([128, KC, 256], F32, tag="nl")
                                nc.scalar.activation(
                                    out=os_, in_=pgB[:, 4:8, :], func=AFT.Sigmoid
                                )
                            gt = npool.tile([128, KC, 256], F32, tag="nl")
                            nc.scalar.activation(
                                out=gt, in_=pgB[:, 0:4, :], func=AFT.Tanh
                            )
                            csl = c[:, :, h0:h0 + w]
                            if t == 0:
                                nc.vector.tensor_mul(csl, si, gt)
                            else:
                                nc.vector.tensor_mul(csl, fs, csl)
                                tmp = npool.tile([128, KC, 256], F32, tag="nl")
                                nc.vector.tensor_mul(tmp, si, gt)
                                nc.vector.tensor_add(csl, csl, tmp)
                            if t < K:
                                tcv = npool.tile([128, KC, 256], F32, tag="nl")
                                nc.scalar.activation(out=tcv, in_=csl, func=AFT.Tanh)
                                nc.vector.tensor_mul(h[:, :, h0:h0 + w], os_, tcv)
                            else:
                                nc.vector.tensor_copy(
                                    out=zout[:, :, h0:h0 + w], in_=csl
                                )
            else:
                with tc.tile_pool(name=f"psg{li}", bufs=2, space="PSUM") as psg:
                    for t in range(K + 1):
                        if t > 0:
                            pg = psg.tile([128, 16, max(m, 16)], F32, tag="pgs")
                            for mm in range(16):
                                if t == K and mm >= 12:
                                    continue
                                for kb in range(KC):
                                    nc.tensor.matmul(
                                        pg[:, mm, 0:m],
                                        whh[:, kb, mm * 128:(mm + 1) * 128],
                                        h[:, kb, 0:m],
                                        start=(kb == 0), stop=(kb == KC - 1),
                                    )
                            gm = 12 if t == K else 16
                            nc.vector.tensor_add(
                                pg[:, 0:gm, 0:m], pg[:, 0:gm, 0:m],
                                gin[:, 0:gm, t * mw:t * mw + m],
                            )
                            gsrc = pg
                            gw = max(m, 16)
                        else:
                            gsrc = gin
                            gw = 9 * max(m, 16)

                        si = npool.tile([128, KC, max(m, 16)], F32, tag="snl")
                        nc.scalar.activation(
                            out=si[:, :, 0:m], in_=gsrc[:, 0:4, 0:m],
                            func=AFT.Sigmoid,
                        )
                        if t > 0:
                            fs = npool.tile([128, KC, max(m, 16)], F32, tag="snl")
                            nc.scalar.activation(
                                out=fs[:, :, 0:m], in_=gsrc[:, 4:8, 0:m],
                                func=AFT.Sigmoid,
                            )
                        if t < K:
                            os_ = npool.tile([128, KC, max(m, 16)], F32, tag="snl")
                            nc.scalar.activation(
                                out=os_[:, :, 0:m], in_=gsrc[:, 12:16, 0:m],
                                func=AFT.Sigmoid,
                            )
                        gt = npool.tile([128, KC, max(m, 16)], F32, tag="snl")
                        nc.scalar.activation(
                            out=gt[:, :, 0:m], in_=gsrc[:, 8:12, 0:m], func=AFT.Tanh
                        )
                        csl = c[:, :, 0:m]
                        if t == 0:
                            nc.vector.tensor_mul(csl, si[:, :, 0:m], gt[:, :, 0:m])
                        else:
                            nc.vector.tensor_mul(csl, fs[:, :, 0:m], csl)
                            tmp = npool.tile([128, KC, max(m, 16)], F32, tag="snl")
                            nc.vector.tensor_mul(
                                tmp[:, :, 0:m], si[:, :, 0:m], gt[:, :, 0:m]
                            )
                            nc.vector.tensor_add(csl, csl, tmp[:, :, 0:m])
                        if t < K:
                            tcv = npool.tile([128, KC, max(m, 16)], F32, tag="snl")
                            nc.scalar.activation(
                                out=tcv[:, :, 0:m], in_=csl, func=AFT.Tanh
                            )
                            nc.vector.tensor_mul(
                                h[:, :, 0:m], os_[:, :, 0:m], tcv[:, :, 0:m]
                            )
                        elif not is_root:
                            nc.vector.tensor_copy(out=zout[:, :, 0:m], in_=csl)

            lvl_stack.close()
            if is_root:
                c_final = c
            zin = zout
            goff += giw[li]
            soff += siw[li]

        nc.sync.dma_start(
            out=out_p.ap().rearrange("a (c p) -> p c a", p=128),
            in_=c_final[:, :, 0:1],
        )

    nc.finalize()
    return nc


def _prep_core_inputs(core, embeddings, dep_types, wih_t_bf, whh_t_bf, wdep_t_bf,
                      bsum_t, bdep_t, caps):
    emb = embeddings

    cap0 = caps[0]
    leaf_emb = emb[OFF[5] + LEAF * core: OFF[5] + LEAF * (core + 1)]
    leaf_dep = dep_types[OFF[5] + LEAF * core: OFF[5] + LEAF * (core + 1)]
    cols = np.zeros(P * cap0, dtype=np.int64)
    for p in range(P):
        idx_p = np.where(leaf_dep == p)[0]
        cols[p * cap0: p * cap0 + len(idx_p)] = idx_p
    xleaf = np.ascontiguousarray(leaf_emb[cols].T).astype(BF)

    xown_cols = []
    for (_, C, m, _, is_root) in LEVEL_SHAPES:
        if is_root:
            xown_cols.append(emb[0:1])
        else:
            lvl = {512: 4, 64: 3, 8: 2, 1: 1}[m]
            s = OFF[lvl] + m * core
            xown_cols.append(emb[s:s + m])
    xown = np.ascontiguousarray(np.concatenate(xown_cols, axis=0).T).astype(BF)

    giw_cols = []
    siw_cols = []
    for li, (_, C, m, _, is_root) in enumerate(LEVEL_SHAPES):
        cap = caps[li]
        mw = _ceil32(m)
        if is_root:
            deps = dep_types[1:9]
        else:
            lvl = {512: 4, 64: 3, 8: 2, 1: 1}[m]
            s = OFF[lvl + 1] + C * core
            deps = dep_types[s:s + C]
        pos_of_child = np.zeros(C, dtype=np.int64)
        for p in range(P):
            idx_p = np.where(deps == p)[0]
            pos_of_child[idx_p] = p * cap + np.arange(len(idx_p))
            if li != 0:
                giw_cols.append(_wrap_idx(idx_p, cap // 16))
        for t in range(K):
            child = np.arange(m) * K + t
            siw_cols.append(_wrap_idx(pos_of_child[child], mw // 16))
    idxg = np.concatenate(giw_cols, axis=1)
    idxs = np.concatenate(siw_cols, axis=1)

    return {
        "xleaf": xleaf,
        "xown": xown,
        "wih_t": wih_t_bf,
        "whh_t": whh_t_bf,
        "wdep_t": wdep_t_bf,
        "bsum_t": bsum_t,
        "bdep_t": bdep_t,
        "idxg": idxg,
        "idxs": idxs,
    }


_CACHED = {}


def kernel(embeddings, dep_types, W_dep, b_dep, W_ih, W_hh, b_ih, b_hh):
    embeddings = np.asarray(embeddings, dtype=np.float32)
    dep_types = np.asarray(dep_types)
    W_dep = np.asarray(W_dep, dtype=np.float32)
    b_dep = np.asarray(b_dep, dtype=np.float32)
    W_ih = np.asarray(W_ih, dtype=np.float32)
    W_hh = np.asarray(W_hh, dtype=np.float32)
    b_ih = np.asarray(b_ih, dtype=np.float32)
    b_hh = np.asarray(b_hh, dtype=np.float32)

    # per-level dep-group capacities (max group size over cores, ceil to 16)
    caps = []
    for (_, C, m, _, is_root) in LEVEL_SHAPES:
        if is_root:
            mx = int(np.bincount(dep_types[1:9], minlength=P).max())
        else:
            lvl = {512: 4, 64: 3, 8: 2, 1: 1}[m]
            mx = 0
            for c in range(NCORES):
                s = OFF[lvl + 1] + C * c
                mx = max(mx, int(np.bincount(dep_types[s:s + C],
                                             minlength=P).max()))
        caps.append(max(_ceil32(mx), 32))

    wih_t_bf = np.ascontiguousarray(W_ih.T).astype(BF)
    whh_t_bf = np.ascontiguousarray(W_hh.T).astype(BF)
    wdep_t_bf = np.ascontiguousarray(W_dep.transpose(0, 2, 1)).astype(BF)
    bsum = (b_ih + b_hh).astype(np.float32)
    bsum_t = np.ascontiguousarray(bsum.reshape(16, 128).T)
    # bdep[:, r*8 + p] = b_dep[p, r*128:(r+1)*128]
    bdep_t = np.ascontiguousarray(
        b_dep.T.reshape(KC, 128, P).transpose(1, 0, 2).reshape(128, KC * P)
    )

    key = tuple(caps)
    if key not in _CACHED:
        _CACHED[key] = build_program(caps)
    nc = _CACHED[key]

    in_maps = [
        _prep_core_inputs(c, embeddings, dep_types, wih_t_bf, whh_t_bf,
                          wdep_t_bf, bsum_t, bdep_t, caps)
        for c in range(NCORES)
    ]
    res = run_bass_kernel_spmd(nc, in_maps, list(range(NCORES)))
    out = np.asarray(res.results[0]["out"], dtype=np.float32).reshape(1, D)
    return out
